# revision 12
# baseline (speedup 1.0000x reference)
"""CharRNN (2-layer miLSTM + big logits GEMM) Trainium2 kernel.

Sharding: data-parallel over batch across 8 cores (4 sequences each).
Each core runs the full T=128 recurrence for its 4 sequences and then
computes logits for its own 512 tokens over the FULL vocab (no
collectives). Host concatenates + row-permutes the 8 shards.

Layout is "transposed": features on partitions, batch on the free dim.
Layer 1 runs SKEW=12 steps behind layer 0; the two layers' per-step
gate math is fused into single double-width instructions using skewed
access patterns over combined (layer, ...) buffers. Matmul inputs are
bf16 (weights pre-cast on host, h stored bf16); gate math is f32.
Logits are produced in bf16 (PSUM bf16 -> big bf16 staging tiles ->
2 MB DMAs); the host upcasts to f32.
"""

import numpy as np
from contextlib import ExitStack

V, E, L, B, T = 32000, 128, 2, 32, 128
G = 4 * E
P = 128
NCORES = 8
BL = B // NCORES          # 4 sequences per core
NTOK = BL * T             # 512 tokens per core
FORGET_BIAS = 1.0
SKEW = 12                 # layer-1 lag (slots = T + SKEW)
ABLK = 8                  # l1 A/C blocklet size (steps)
NB = 4                    # logits blocks (128 tokens each)
SPB = T // NB             # steps per logits block = 32
TPB = SPB * BL            # tokens per logits block = 128
NTW = 500                 # logits n-tile width
NPT = V // NTW            # 64 n-tiles per block
CHW = 8000                # out DMA chunk width (16 n-tiles)
NCH = V // CHW            # 4 chunks per block

_cache = {}


def _build(use_smax_bias):
    import concourse.bass as bass
    import concourse.tile as tile
    import concourse.mybir as mybir
    from concourse import bacc
    from concourse.bass import IndirectOffsetOnAxis
    from concourse.masks import make_identity

    dt = mybir.dt
    AF = mybir.ActivationFunctionType
    OP = mybir.AluOpType

    nc = bacc.Bacc("TRN2", target_bir_lowering=False, debug=False,
                   num_devices=NCORES)

    ids_d = nc.dram_tensor("ids", (P, BL), dt.int32, kind="ExternalInput")
    emb_d = nc.dram_tensor("emb", (V, E), dt.float32, kind="ExternalInput")
    wxa_d = nc.dram_tensor("wxa", (P, L, G), dt.bfloat16, kind="ExternalInput")
    wxc_d = nc.dram_tensor("wxc", (P, L, G), dt.bfloat16, kind="ExternalInput")
    wh_d = nc.dram_tensor("wh", (P, L, G), dt.bfloat16, kind="ExternalInput")
    b2t_d = nc.dram_tensor("b2t", (P, L, 4), dt.float32, kind="ExternalInput")
    bft_d = nc.dram_tensor("bft", (P, L, 4), dt.float32, kind="ExternalInput")
    pep_d = nc.dram_tensor("pep", (P, L, 3), dt.float32, kind="ExternalInput")
    wbif_d = nc.dram_tensor("wbif", (P, L, 3, BL), dt.float32,
                            kind="ExternalInput")
    wbo_d = nc.dram_tensor("wbo", (P, L, BL), dt.float32,
                           kind="ExternalInput")
    swt_d = nc.dram_tensor("swt", (P, V), dt.bfloat16, kind="ExternalInput")
    if use_smax_bias:
        smb_d = nc.dram_tensor("smb", (1, V), dt.float32, kind="ExternalInput")
    # rows of out are in device token order (t*BL + s); host un-permutes
    out_d = nc.dram_tensor("out", (NTOK, V), dt.bfloat16,
                           kind="ExternalOutput")

    with tile.TileContext(nc) as tc, ExitStack() as ctx:
        singles = ctx.enter_context(tc.tile_pool(name="singles", bufs=1))
        big = ctx.enter_context(tc.tile_pool(name="big", bufs=1))
        stage_p = ctx.enter_context(tc.tile_pool(name="stage", bufs=3))
        rec = ctx.enter_context(tc.tile_pool(name="rec", bufs=3))
        cpool = ctx.enter_context(tc.tile_pool(name="cpool", bufs=3))
        ps_big = ctx.enter_context(
            tc.tile_pool(name="ps_big", bufs=2, space="PSUM"))
        ps_g = ctx.enter_context(
            tc.tile_pool(name="ps_g", bufs=3, space="PSUM"))
        ps_log = ctx.enter_context(
            tc.tile_pool(name="ps_log", bufs=3, space="PSUM"))

        # ---- static inputs -> SBUF ----
        ids_sb = singles.tile([P, BL], dt.int32)
        nc.sync.dma_start(out=ids_sb[:, :], in_=ids_d[:, :])
        wxa_sb = singles.tile([P, L, G], dt.bfloat16)
        nc.sync.dma_start(out=wxa_sb[:, :, :], in_=wxa_d[:, :, :])
        wxc_sb = singles.tile([P, L, G], dt.bfloat16)
        nc.sync.dma_start(out=wxc_sb[:, :, :], in_=wxc_d[:, :, :])
        wh_sb = singles.tile([P, L, G], dt.bfloat16)
        nc.sync.dma_start(out=wh_sb[:, :, :], in_=wh_d[:, :, :])
        b2t_sb = singles.tile([P, L, 4], dt.float32)
        nc.sync.dma_start(out=b2t_sb[:, :, :], in_=b2t_d[:, :, :])
        bft_sb = singles.tile([P, L, 4], dt.float32)
        nc.sync.dma_start(out=bft_sb[:, :, :], in_=bft_d[:, :, :])
        pep_sb = singles.tile([P, L, 3], dt.float32)
        nc.sync.dma_start(out=pep_sb[:, :, :], in_=pep_d[:, :, :])
        wbif_sb = singles.tile([P, L, 3, BL], dt.float32)
        nc.sync.dma_start(out=wbif_sb[:, :, :, :], in_=wbif_d[:, :, :, :])
        wbo_sb = singles.tile([P, L, BL], dt.float32)
        nc.sync.dma_start(out=wbo_sb[:, :, :], in_=wbo_d[:, :, :])
        swt_sb = singles.tile([P, V], dt.bfloat16)
        for q in range(8):
            nc.sync.dma_start(out=swt_sb[:, q * 4000:(q + 1) * 4000],
                              in_=swt_d[:, q * 4000:(q + 1) * 4000])
        if use_smax_bias:
            smb_sb = singles.tile([1, V], dt.float32)
            nc.sync.dma_start(out=smb_sb[:, :], in_=smb_d[:, :])
            ones1 = singles.tile([1, P], dt.float32)
            nc.vector.memset(ones1[:, :], 1.0)

        ident = singles.tile([P, P], dt.float32)
        make_identity(nc, ident[:, :])

        zeros4 = singles.tile([P, BL], dt.float32)
        nc.vector.memset(zeros4[:, :], 0.0)
        zeros4h = singles.tile([P, BL], dt.bfloat16)
        nc.vector.memset(zeros4h[:, :], 0.0)

        # ---- embedding gather (tokens on partitions) + transpose ----
        x_sb = singles.tile([P, BL, E], dt.float32)
        for m in range(BL):
            nc.gpsimd.indirect_dma_start(
                out=x_sb[:, m, :], out_offset=None,
                in_=emb_d[:, :],
                in_offset=IndirectOffsetOnAxis(ap=ids_sb[:, m:m + 1], axis=0),
            )
        xT = singles.tile([P, NTOK], dt.bfloat16)
        for m in range(BL):
            pst = ps_big.tile([P, P], dt.float32, tag="psac")
            nc.tensor.transpose(pst[:, :], x_sb[:, m, :], ident[:, :])
            nc.scalar.copy(xT[:, m * P:(m + 1) * P], pst[:, :])

        # ---- combined (layer, ...) buffers ----
        a_all = big.tile([P, L, 4, NTOK], dt.float32)
        c_all = big.tile([P, L, 4, NTOK], dt.float32)
        hT = big.tile([P, L, NTOK], dt.bfloat16)

        SKL_A = a_all.ap[1][0] - SKEW * BL    # layer stride minus skew
        SKL_H = hT.ap[1][0] - SKEW * BL

        def a_skew(t):
            return bass.AP(a_all.tensor, a_all.offset + t * BL,
                           [a_all.ap[0], [SKL_A, 2], a_all.ap[2], [1, BL]])

        def c_skew(t):
            return bass.AP(c_all.tensor, c_all.offset + t * BL,
                           [c_all.ap[0], [SKL_A, 2], c_all.ap[2], [1, BL]])

        def h_skew(t):
            return bass.AP(hT.tensor, hT.offset + t * BL,
                           [hT.ap[0], [SKL_H, 2], [1, BL]])

        def c_bcast3(cp):  # (P, 2, BL) c-slice -> (P, 2, 3, BL), dup gate dim
            return bass.AP(cp.tensor, cp.offset,
                           [cp.ap[0], cp.ap[1], [0, 3], cp.ap[2]])

        def emit_ac(l, tok0, ntok):
            src = xT if l == 0 else hT[:, 0, :]
            blk = slice(tok0, tok0 + ntok)
            for k in range(4):
                psa = ps_big.tile([P, TPB], dt.float32, tag="psac")
                nc.tensor.matmul(psa[:, 0:ntok],
                                 wxa_sb[:, l, k * P:(k + 1) * P],
                                 src[:, blk])
                nc.scalar.activation(a_all[:, l, k, blk], psa[:, 0:ntok],
                                     AF.Identity, bias=b2t_sb[:, l, k:k + 1])
                psc = ps_big.tile([P, TPB], dt.float32, tag="psac")
                nc.tensor.matmul(psc[:, 0:ntok],
                                 wxc_sb[:, l, k * P:(k + 1) * P],
                                 src[:, blk])
                nc.vector.tensor_scalar_add(c_all[:, l, k, blk],
                                            psc[:, 0:ntok],
                                            bft_sb[:, l, k:k + 1])

        # recurrence state: cnt tile (P, 2, 3, BL); slot 0 = next step's
        # tanh'(g_j), slot 1 = c_t, slot 2 = o2h_t. j-gate a/c are host-
        # doubled (tanh(x) = 2*sigmoid(2x)-1); o-gate a/c + wo host-halved
        # (sigmoid(o2) = sigmoid(2*o2h)).
        cnt_prev = None
        h_prev = [zeros4h[:, :], zeros4h[:, :]]

        def emit_step_single(l, t, zero_other=False):
            # one-layer step (pipeline head/tail); state kept in pair tiles
            nonlocal cnt_prev
            tb = slice(t * BL, (t + 1) * BL)
            psg = ps_g.tile([P, 2, 4, BL], dt.float32, tag="psg")
            for k in range(4):
                nc.tensor.matmul(psg[:, l, k, :],
                                 wh_sb[:, l, k * P:(k + 1) * P],
                                 h_prev[l], start=(k == 0), stop=(k == 3),
                                 skip_group_check=True)
            cp = zeros4[:, :] if cnt_prev is None else cnt_prev[:, l, 1, :]
            g = rec.tile([P, 4, BL], dt.float32, tag="g")
            nc.vector.tensor_tensor(g[:, :, :], psg[:, l, :, :],
                                    a_all[:, l, :, tb], op=OP.mult)
            nc.vector.tensor_tensor(g[:, :, :], g[:, :, :],
                                    c_all[:, l, :, tb], op=OP.add)
            if2 = rec.tile([P, 2, BL], dt.float32, tag="if2")
            nc.vector.scalar_tensor_tensor(
                if2[:, 0, :], cp, pep_sb[:, l, 0:1], g[:, 0, :],
                op0=OP.mult, op1=OP.add)
            nc.vector.scalar_tensor_tensor(
                if2[:, 1, :], cp, pep_sb[:, l, 1:2], g[:, 1, :],
                op0=OP.mult, op1=OP.add)
            sif = rec.tile([P, 2, BL], dt.float32, tag="sif")
            nc.scalar.activation(sif[:, :, :], if2[:, :, :], AF.Sigmoid)
            # g_j is host-doubled: tanh(true j) = tanh(g_j / 2)
            tj = rec.tile([P, BL], dt.float32, tag="tj")
            nc.scalar.activation(tj[:, :], g[:, 2, :], AF.Tanh, scale=0.5)
            u = rec.tile([P, BL], dt.float32, tag="u")
            nc.vector.tensor_tensor(u[:, :], sif[:, 0, :], tj[:, :],
                                    op=OP.mult)
            v = rec.tile([P, BL], dt.float32, tag="v")
            nc.vector.tensor_tensor(v[:, :], sif[:, 1, :], cp, op=OP.mult)
            cn = cpool.tile([P, 2, 3, BL], dt.float32, tag="cn")
            nc.vector.tensor_tensor(cn[:, l, 1, :], u[:, :], v[:, :],
                                    op=OP.add)
            if zero_other:
                nc.vector.memset(cn[:, 1 - l, 1, :], 0.0)
            # o-gate host-halved: o2h = o2/2; so = sigmoid(2*o2h)
            o2 = rec.tile([P, BL], dt.float32, tag="o2")
            nc.vector.scalar_tensor_tensor(
                o2[:, :], cn[:, l, 1, :], pep_sb[:, l, 2:3], g[:, 3, :],
                op0=OP.mult, op1=OP.add)
            so = rec.tile([P, BL], dt.float32, tag="so")
            nc.scalar.activation(so[:, :], o2[:, :], AF.Sigmoid, scale=2.0)
            tc_ = rec.tile([P, BL], dt.float32, tag="tc")
            nc.scalar.activation(tc_[:, :], cn[:, l, 1, :], AF.Tanh)
            nc.vector.tensor_tensor(hT[:, l, tb], so[:, :], tc_[:, :],
                                    op=OP.mult)
            cnt_prev = cn[:, :, :, :]
            h_prev[l] = hT[:, l, tb]

        def emit_pair(t0):
            # fused: layer0 step t0 + layer1 step t0-SKEW
            nonlocal cnt_prev
            t1 = t0 - SKEW
            psg = ps_g.tile([P, 2, 4, BL], dt.float32, tag="psg")
            for li, tt_ in ((0, t0), (1, t1)):
                for k in range(4):
                    nc.tensor.matmul(
                        psg[:, li, k, :], wh_sb[:, li, k * P:(k + 1) * P],
                        h_prev[li], start=(li == 0 and k == 0),
                        stop=(li == 1 and k == 3), skip_group_check=True)
            cpt = cnt_prev
            g = rec.tile([P, 2, 4, BL], dt.float32, tag="gp")
            nc.vector.tensor_tensor(g[:, :, :, :], psg[:, :, :, :],
                                    a_skew(t0), op=OP.mult)
            nc.vector.tensor_tensor(g[:, :, :, :], g[:, :, :, :],
                                    c_skew(t0), op=OP.add)
            # wic3 = [c*wi, c*wf, 0]; if2j = g[ifj] + wic3 = [if2_i, if2_f, gj]
            wic = rec.tile([P, 2, 3, BL], dt.float32, tag="wic")
            nc.vector.tensor_tensor(wic[:, :, :, :], c_bcast3(cpt[:, :, 1, :]),
                                    wbif_sb[:, :, :, :], op=OP.mult)
            if2 = rec.tile([P, 2, 3, BL], dt.float32, tag="if2p")
            nc.vector.tensor_tensor(if2[:, :, :, :], wic[:, :, :, :],
                                    g[:, :, 0:3, :], op=OP.add)
            # S = [sig(if2_i), sig(if2_f), sig(2*j_true)]
            sif = rec.tile([P, 2, 3, BL], dt.float32, tag="sifp")
            nc.scalar.activation(sif[:, :, :, :], if2[:, :, :, :], AF.Sigmoid)
            # tanh'(j) = 2*sig(2j)-1 written into prev cnt slot 0
            nc.vector.tensor_scalar(out=cpt[:, :, 0, :],
                                    in0=sif[:, :, 2, :], scalar1=2.0,
                                    scalar2=-1.0, op0=OP.mult, op1=OP.add)
            # [u|v] = [sig_i*tanh_j | sig_f*c_prev] in one op
            uv = rec.tile([P, 2, 2, BL], dt.float32, tag="uvp")
            nc.vector.tensor_tensor(uv[:, :, :, :], sif[:, :, 0:2, :],
                                    cpt[:, :, 0:2, :], op=OP.mult)
            cn = cpool.tile([P, 2, 3, BL], dt.float32, tag="cn")
            nc.vector.tensor_tensor(cn[:, :, 1, :], uv[:, :, 0, :],
                                    uv[:, :, 1, :], op=OP.add)
            woc = rec.tile([P, 2, BL], dt.float32, tag="wop")
            nc.vector.tensor_tensor(woc[:, :, :], cn[:, :, 1, :],
                                    wbo_sb[:, :, :], op=OP.mult)
            nc.vector.tensor_tensor(cn[:, :, 2, :], woc[:, :, :],
                                    g[:, :, 3, :], op=OP.add)
            # S2 = [sig(2c) | sig(2*o2h)] = [tanh-core | so]
            s2 = rec.tile([P, 2, 2, BL], dt.float32, tag="s2p")
            nc.scalar.activation(s2[:, :, :, :], cn[:, :, 1:3, :],
                                 AF.Sigmoid, scale=2.0)
            tc_ = rec.tile([P, 2, BL], dt.float32, tag="tcp")
            nc.vector.tensor_scalar(out=tc_[:, :, :], in0=s2[:, :, 0, :],
                                    scalar1=2.0, scalar2=-1.0,
                                    op0=OP.mult, op1=OP.add)
            nc.vector.tensor_tensor(h_skew(t0), s2[:, :, 1, :], tc_[:, :, :],
                                    op=OP.mult)
            cnt_prev = cn[:, :, :, :]
            h_prev[0] = hT[:, 0, t0 * BL:(t0 + 1) * BL]
            h_prev[1] = hT[:, 1, t1 * BL:(t1 + 1) * BL]

        # logits staging: one chunk = 16 n-tiles of 500 -> 8000 cols
        cur_st = [None]

        def emit_logits_ntile(k, n, eng):
            n0 = n * NTW
            q = n // 16
            if n % 16 == 0:
                cur_st[0] = stage_p.tile([P, CHW], dt.bfloat16, tag="st",
                                         name="st")
            st = cur_st[0]
            c0 = n0 - q * CHW
            ps = ps_log.tile([P, NTW], dt.float32)
            nc.tensor.matmul(ps[:, :], hT[:, 1, k * TPB:(k + 1) * TPB],
                             swt_sb[:, n0:n0 + NTW],
                             start=True, stop=not use_smax_bias)
            if use_smax_bias:
                nc.tensor.matmul(ps[:, :], ones1[:, :],
                                 smb_sb[:, n0:n0 + NTW],
                                 start=False, stop=True)
            if eng == 0:
                nc.vector.tensor_copy(st[:, c0:c0 + NTW], ps[:, :])
            else:
                nc.scalar.copy(st[:, c0:c0 + NTW], ps[:, :])
            if n % 16 == 15:
                nc.sync.dma_start(
                    out=out_d[k * TPB:(k + 1) * TPB, q * CHW:(q + 1) * CHW],
                    in_=st[:, :])

        # layer-0 A/C for all tokens (x fully available)
        for j in range(NB):
            emit_ac(0, j * TPB, TPB)

        # ---- pipelined recurrence + logits ----
        NSLOT = T + SKEW
        pending = []
        ne = 0
        for s in range(NSLOT):
            if s < SKEW:
                emit_step_single(0, s, zero_other=(s == SKEW - 1))
            elif s < T:
                emit_pair(s)
            else:
                emit_step_single(1, s - SKEW)
            # l1 A/C blocklets: blocklet j (tokens 8j..8j+8) after slot 8j+7
            if s >= ABLK - 1 and (s - (ABLK - 1)) % ABLK == 0:
                j = (s - (ABLK - 1)) // ABLK
                if j < T // ABLK:
                    emit_ac(1, j * ABLK * BL, ABLK * BL)
            # logits block k ready after slot 32k+31+SKEW
            if s >= SPB - 1 + SKEW and (s - (SPB - 1) - SKEW) % SPB == 0:
                k = (s - (SPB - 1) - SKEW) // SPB
                if k < NB:
                    pending.extend(((k, n) for n in range(NPT)))
            for _ in range(2 if s % 2 == 0 else 3):
                if ne < len(pending):
                    k, n = pending[ne]
                    emit_logits_ntile(k, n, 0 if ne % 3 == 0 else 1)
                    ne += 1
        while ne < len(pending):
            k, n = pending[ne]
            emit_logits_ntile(k, n, 0 if ne % 3 == 0 else 1)
            ne += 1

    nc.compile()
    return nc


def _prep_inputs(input_data, embedding, Wx, Wh, alpha, beta1, beta2, bias,
                 wi, wf, wo, softmax_w, softmax_b):
    import ml_dtypes
    bf16 = ml_dtypes.bfloat16
    f32 = np.float32
    input_data = np.asarray(input_data, np.int32)
    embedding = np.ascontiguousarray(np.asarray(embedding, f32))
    Wx = np.asarray(Wx, f32)
    Wh = np.asarray(Wh, f32)
    alpha = np.asarray(alpha, f32)
    beta1 = np.asarray(beta1, f32)
    beta2 = np.asarray(beta2, f32)
    bias = np.asarray(bias, f32)
    wi = np.asarray(wi, f32)
    wf = np.asarray(wf, f32)
    wo = np.asarray(wo, f32)
    softmax_w = np.asarray(softmax_w, f32)
    softmax_b = np.asarray(softmax_b, f32)

    gperm = [0, 2, 1, 3]   # reference order i,j,f,o -> device order i,f,j,o

    def permG(a):
        r = a.reshape(*a.shape[:-1], 4, E)
        return np.ascontiguousarray(r[..., gperm, :].reshape(*a.shape))

    WxA = permG(Wx * alpha[:, None, :]).copy()
    WxC = permG(Wx * beta1[:, None, :]).copy()
    Whp = permG(Wh)
    b2p = permG(beta2).copy()
    bp = permG(bias).copy()
    bp[:, E:2 * E] += FORGET_BIAS          # f-chunk in [i|f|j|o] order
    # j-gate doubled (tanh via 2*sig(2x)-1), o-gate halved (sig(2*o2h))
    jsl, osl = slice(2 * E, 3 * E), slice(3 * E, 4 * E)
    for arr in (WxA, WxC):
        arr[:, :, jsl] *= 2.0
        arr[:, :, osl] *= 0.5
    for arr in (b2p, bp):
        arr[:, jsl] *= 2.0
        arr[:, osl] *= 0.5

    def to_elg(a):
        return np.ascontiguousarray(np.transpose(a, (1, 0, 2)))

    def to_plk(a):
        return np.ascontiguousarray(
            np.transpose(a.reshape(L, 4, E), (2, 0, 1)))

    pep = np.ascontiguousarray(
        np.transpose(np.stack([wi, wf, wo * 0.5], axis=1),
                     (2, 0, 1)))  # (E, L, 3)
    wbif = np.ascontiguousarray(np.broadcast_to(
        np.transpose(np.stack([wi, wf, np.zeros_like(wi)], axis=1),
                     (2, 0, 1))[:, :, :, None],
        (E, L, 3, BL))).astype(f32)
    wbo = np.ascontiguousarray(np.broadcast_to(
        (wo * 0.5).T[:, :, None], (E, L, BL))).astype(f32)

    swt = np.ascontiguousarray(softmax_w.T)
    use_smax_bias = bool(np.any(softmax_b))

    common = {
        "emb": embedding,
        "wxa": to_elg(WxA).astype(bf16), "wxc": to_elg(WxC).astype(bf16),
        "wh": to_elg(Whp).astype(bf16),
        "b2t": to_plk(b2p), "bft": to_plk(bp), "pep": pep,
        "wbif": wbif, "wbo": wbo,
        "swt": swt.astype(bf16),
    }
    if use_smax_bias:
        common["smb"] = softmax_b.reshape(1, V)

    tok = np.arange(NTOK)
    tt_, ss_ = tok // BL, tok % BL
    in_maps = []
    for c in range(NCORES):
        flat = input_data[BL * c + ss_, tt_]
        ids_pm = np.ascontiguousarray(flat.reshape(BL, P).T.astype(np.int32))
        in_maps.append({"ids": ids_pm, **common})
    return in_maps, use_smax_bias


def _run(in_maps, use_smax_bias, trace=False, tmpdir=None):
    from concourse.bass_utils import run_bass_kernel_spmd
    key = use_smax_bias
    if key not in _cache:
        _cache[key] = _build(use_smax_bias)
    nc = _cache[key]
    return run_bass_kernel_spmd(nc, in_maps, core_ids=list(range(NCORES)),
                                trace=trace, tmpdir=tmpdir)


def kernel(**inputs):
    in_maps, use_smax_bias = _prep_inputs(**inputs)
    res = _run(in_maps, use_smax_bias, trace=False)
    # device rows are token order (t*BL + s); reference rows are s*T + t
    tok = np.arange(NTOK)
    row = (tok % BL) * T + tok // BL
    out = np.empty((B * T, V), np.float32)
    for c in range(NCORES):
        out[c * NTOK + row] = res.results[c]["out"].astype(np.float32)
    return out


# revision 15
# speedup vs baseline: 1.0884x; 1.0884x over previous
"""CharRNN (2-layer miLSTM + big logits GEMM) Trainium2 kernel.

Sharding: data-parallel over batch across 8 cores (4 sequences each).
Each core runs the full T=128 recurrence for its 4 sequences and then
computes logits for its own 512 tokens over the FULL vocab (no
collectives). Host concatenates + row-permutes the 8 shards.

Layout is "transposed": features on partitions, batch on the free dim.
Layer 1 runs SKEW=12 steps behind layer 0; the two layers' per-step
gate math is fused into single double-width instructions using skewed
access patterns over combined (layer, ...) buffers. Matmul inputs are
bf16 (weights pre-cast on host, h stored bf16); gate math is f32.
Logits are produced in bf16 (PSUM bf16 -> big bf16 staging tiles ->
2 MB DMAs); the host upcasts to f32.
"""

import numpy as np
from contextlib import ExitStack

V, E, L, B, T = 32000, 128, 2, 32, 128
G = 4 * E
P = 128
NCORES = 8
BL = B // NCORES          # 4 sequences per core
NTOK = BL * T             # 512 tokens per core
FORGET_BIAS = 1.0
SKEW = 12                 # layer-1 lag (slots = T + SKEW)
ABLK = 8                  # l1 A/C blocklet size (steps)
NB = 4                    # logits blocks (128 tokens each)
SPB = T // NB             # steps per logits block = 32
TPB = SPB * BL            # tokens per logits block = 128
NTW = 500                 # logits n-tile width
NPT = V // NTW            # 64 n-tiles per block
CHW = 8000                # out DMA chunk width (16 n-tiles)
NCH = V // CHW            # 4 chunks per block

_cache = {}


def _build(use_smax_bias):
    import concourse.bass as bass
    import concourse.tile as tile
    import concourse.mybir as mybir
    from concourse import bacc
    from concourse.bass import IndirectOffsetOnAxis
    from concourse.masks import make_identity

    dt = mybir.dt
    AF = mybir.ActivationFunctionType
    OP = mybir.AluOpType

    nc = bacc.Bacc("TRN2", target_bir_lowering=False, debug=False,
                   num_devices=NCORES)

    ids_d = nc.dram_tensor("ids", (P, BL), dt.int32, kind="ExternalInput")
    emb_d = nc.dram_tensor("emb", (V, E), dt.float32, kind="ExternalInput")
    wx_d = nc.dram_tensor("wx", (P, L, G), dt.bfloat16, kind="ExternalInput")
    alp_d = nc.dram_tensor("alp", (P, L, 4), dt.float32, kind="ExternalInput")
    b1t_d = nc.dram_tensor("b1t", (P, L, 4), dt.float32, kind="ExternalInput")
    wh_d = nc.dram_tensor("wh", (P, L, G), dt.bfloat16, kind="ExternalInput")
    b2t_d = nc.dram_tensor("b2t", (P, L, 4), dt.float32, kind="ExternalInput")
    bft_d = nc.dram_tensor("bft", (P, L, 4), dt.float32, kind="ExternalInput")
    pep_d = nc.dram_tensor("pep", (P, L, 3), dt.float32, kind="ExternalInput")
    wbif_d = nc.dram_tensor("wbif", (P, L, 2, BL), dt.float32,
                            kind="ExternalInput")
    wbo_d = nc.dram_tensor("wbo", (P, L, BL), dt.float32,
                           kind="ExternalInput")
    swt_d = nc.dram_tensor("swt", (P, V), dt.bfloat16, kind="ExternalInput")
    if use_smax_bias:
        smb_d = nc.dram_tensor("smb", (1, V), dt.float32, kind="ExternalInput")
    # rows of out are in device token order (t*BL + s); host un-permutes
    out_d = nc.dram_tensor("out", (NTOK, V), dt.bfloat16,
                           kind="ExternalOutput")

    with tile.TileContext(nc) as tc, ExitStack() as ctx:
        singles = ctx.enter_context(tc.tile_pool(name="singles", bufs=1))
        big = ctx.enter_context(tc.tile_pool(name="big", bufs=1))
        stage_p = ctx.enter_context(tc.tile_pool(name="stage", bufs=3))
        rec = ctx.enter_context(tc.tile_pool(name="rec", bufs=3))
        cpool = ctx.enter_context(tc.tile_pool(name="cpool", bufs=3))
        ps_big = ctx.enter_context(
            tc.tile_pool(name="ps_big", bufs=2, space="PSUM"))
        ps_g = ctx.enter_context(
            tc.tile_pool(name="ps_g", bufs=3, space="PSUM"))
        ps_log = ctx.enter_context(
            tc.tile_pool(name="ps_log", bufs=3, space="PSUM"))

        # ---- static inputs -> SBUF ----
        ids_sb = singles.tile([P, BL], dt.int32)
        nc.sync.dma_start(out=ids_sb[:, :], in_=ids_d[:, :])
        wx_sb = singles.tile([P, L, G], dt.bfloat16)
        nc.sync.dma_start(out=wx_sb[:, :, :], in_=wx_d[:, :, :])
        alp_sb = singles.tile([P, L, 4], dt.float32)
        nc.sync.dma_start(out=alp_sb[:, :, :], in_=alp_d[:, :, :])
        b1t_sb = singles.tile([P, L, 4], dt.float32)
        nc.sync.dma_start(out=b1t_sb[:, :, :], in_=b1t_d[:, :, :])
        wh_sb = singles.tile([P, L, G], dt.bfloat16)
        nc.sync.dma_start(out=wh_sb[:, :, :], in_=wh_d[:, :, :])
        b2t_sb = singles.tile([P, L, 4], dt.float32)
        nc.sync.dma_start(out=b2t_sb[:, :, :], in_=b2t_d[:, :, :])
        bft_sb = singles.tile([P, L, 4], dt.float32)
        nc.sync.dma_start(out=bft_sb[:, :, :], in_=bft_d[:, :, :])
        pep_sb = singles.tile([P, L, 3], dt.float32)
        nc.sync.dma_start(out=pep_sb[:, :, :], in_=pep_d[:, :, :])
        wbif_sb = singles.tile([P, L, 2, BL], dt.float32)
        nc.sync.dma_start(out=wbif_sb[:, :, :, :], in_=wbif_d[:, :, :, :])
        wbo_sb = singles.tile([P, L, BL], dt.float32)
        nc.sync.dma_start(out=wbo_sb[:, :, :], in_=wbo_d[:, :, :])
        swt_sb = singles.tile([P, V], dt.bfloat16)
        for q in range(8):
            nc.sync.dma_start(out=swt_sb[:, q * 4000:(q + 1) * 4000],
                              in_=swt_d[:, q * 4000:(q + 1) * 4000])
        if use_smax_bias:
            smb_sb = singles.tile([1, V], dt.float32)
            nc.sync.dma_start(out=smb_sb[:, :], in_=smb_d[:, :])
            ones1 = singles.tile([1, P], dt.float32)
            nc.vector.memset(ones1[:, :], 1.0)

        ident = singles.tile([P, P], dt.float32)
        make_identity(nc, ident[:, :])

        zeros4 = singles.tile([P, BL], dt.float32)
        nc.vector.memset(zeros4[:, :], 0.0)
        zeros4h = singles.tile([P, BL], dt.bfloat16)
        nc.vector.memset(zeros4h[:, :], 0.0)

        # ---- embedding gather (tokens on partitions) + transpose ----
        x_sb = singles.tile([P, BL, E], dt.float32)
        for m in range(BL):
            nc.gpsimd.indirect_dma_start(
                out=x_sb[:, m, :], out_offset=None,
                in_=emb_d[:, :],
                in_offset=IndirectOffsetOnAxis(ap=ids_sb[:, m:m + 1], axis=0),
            )
        xT = singles.tile([P, NTOK], dt.bfloat16)
        for m in range(BL):
            pst = ps_big.tile([P, P], dt.float32, tag="psac")
            nc.tensor.transpose(pst[:, :], x_sb[:, m, :], ident[:, :])
            nc.scalar.copy(xT[:, m * P:(m + 1) * P], pst[:, :])

        # ---- combined (layer, ...) buffers ----
        a_all = big.tile([P, L, 4, NTOK], dt.float32)
        c_all = big.tile([P, L, 4, NTOK], dt.float32)
        hT = big.tile([P, L, NTOK], dt.bfloat16)

        SKL_A = a_all.ap[1][0] - SKEW * BL    # layer stride minus skew
        SKL_H = hT.ap[1][0] - SKEW * BL

        def a_skew(t):
            return bass.AP(a_all.tensor, a_all.offset + t * BL,
                           [a_all.ap[0], [SKL_A, 2], a_all.ap[2], [1, BL]])

        def c_skew(t):
            return bass.AP(c_all.tensor, c_all.offset + t * BL,
                           [c_all.ap[0], [SKL_A, 2], c_all.ap[2], [1, BL]])

        def h_skew(t):
            return bass.AP(hT.tensor, hT.offset + t * BL,
                           [hT.ap[0], [SKL_H, 2], [1, BL]])

        def c_bcast(cp):  # (P, 2, BL) pair-c -> (P, 2, 2, BL), dup gate dim
            return bass.AP(cp.tensor, cp.offset,
                           [cp.ap[0], cp.ap[1], [0, 2], cp.ap[2]])

        def emit_ac(l, tok0, ntok):
            src = xT if l == 0 else hT[:, 0, :]
            blk = slice(tok0, tok0 + ntok)
            for k in range(4):
                psx = ps_big.tile([P, TPB], dt.float32, tag="psac")
                nc.tensor.matmul(psx[:, 0:ntok],
                                 wx_sb[:, l, k * P:(k + 1) * P],
                                 src[:, blk])
                nc.vector.tensor_scalar(
                    out=a_all[:, l, k, blk], in0=psx[:, 0:ntok],
                    scalar1=alp_sb[:, l, k:k + 1],
                    scalar2=b2t_sb[:, l, k:k + 1],
                    op0=OP.mult, op1=OP.add)
                nc.scalar.activation(c_all[:, l, k, blk], psx[:, 0:ntok],
                                     AF.Identity,
                                     bias=bft_sb[:, l, k:k + 1],
                                     scale=b1t_sb[:, l, k:k + 1])

        # recurrence state
        cpair_prev = None          # AP (P, 2, BL): [c0_t, c1_{t-SKEW}]
        h_prev = [zeros4h[:, :], zeros4h[:, :]]

        def emit_step_single(l, t, zero_other=False):
            # one-layer step (pipeline head/tail); state kept in pair tiles
            nonlocal cpair_prev
            tb = slice(t * BL, (t + 1) * BL)
            psg = ps_g.tile([P, 2, 4, BL], dt.float32, tag="psg")
            for k in range(4):
                nc.tensor.matmul(psg[:, l, k, :],
                                 wh_sb[:, l, k * P:(k + 1) * P],
                                 h_prev[l], start=(k == 0), stop=(k == 3),
                                 skip_group_check=True)
            cp = zeros4[:, :] if cpair_prev is None else cpair_prev[:, l, :]
            g = rec.tile([P, 4, BL], dt.float32, tag="g")
            nc.vector.tensor_tensor(g[:, :, :], psg[:, l, :, :],
                                    a_all[:, l, :, tb], op=OP.mult)
            nc.vector.tensor_tensor(g[:, :, :], g[:, :, :],
                                    c_all[:, l, :, tb], op=OP.add)
            if2 = rec.tile([P, 2, BL], dt.float32, tag="if2")
            nc.vector.scalar_tensor_tensor(
                if2[:, 0, :], cp, pep_sb[:, l, 0:1], g[:, 0, :],
                op0=OP.mult, op1=OP.add)
            nc.vector.scalar_tensor_tensor(
                if2[:, 1, :], cp, pep_sb[:, l, 1:2], g[:, 1, :],
                op0=OP.mult, op1=OP.add)
            sif = rec.tile([P, 2, BL], dt.float32, tag="sif")
            nc.scalar.activation(sif[:, :, :], if2[:, :, :], AF.Sigmoid)
            tj = rec.tile([P, BL], dt.float32, tag="tj")
            nc.scalar.activation(tj[:, :], g[:, 2, :], AF.Tanh)
            u = rec.tile([P, BL], dt.float32, tag="u")
            nc.vector.tensor_tensor(u[:, :], sif[:, 0, :], tj[:, :],
                                    op=OP.mult)
            v = rec.tile([P, BL], dt.float32, tag="v")
            nc.vector.tensor_tensor(v[:, :], sif[:, 1, :], cp, op=OP.mult)
            cn = cpool.tile([P, 2, BL], dt.float32, tag="cn")
            nc.vector.tensor_tensor(cn[:, l, :], u[:, :], v[:, :], op=OP.add)
            if zero_other:
                nc.vector.memset(cn[:, 1 - l, :], 0.0)
            o2 = rec.tile([P, BL], dt.float32, tag="o2")
            nc.vector.scalar_tensor_tensor(
                o2[:, :], cn[:, l, :], pep_sb[:, l, 2:3], g[:, 3, :],
                op0=OP.mult, op1=OP.add)
            so = rec.tile([P, BL], dt.float32, tag="so")
            nc.scalar.activation(so[:, :], o2[:, :], AF.Sigmoid)
            tc_ = rec.tile([P, BL], dt.float32, tag="tc")
            nc.scalar.activation(tc_[:, :], cn[:, l, :], AF.Tanh)
            nc.vector.tensor_tensor(hT[:, l, tb], so[:, :], tc_[:, :],
                                    op=OP.mult)
            cpair_prev = cn[:, :, :]
            h_prev[l] = hT[:, l, tb]

        def emit_pair(t0):
            # fused: layer0 step t0 + layer1 step t0-SKEW
            nonlocal cpair_prev
            t1 = t0 - SKEW
            psg = ps_g.tile([P, 2, 4, BL], dt.float32, tag="psg")
            for li, tt_ in ((0, t0), (1, t1)):
                for k in range(4):
                    nc.tensor.matmul(
                        psg[:, li, k, :], wh_sb[:, li, k * P:(k + 1) * P],
                        h_prev[li], start=(li == 0 and k == 0),
                        stop=(li == 1 and k == 3), skip_group_check=True)
            cp = cpair_prev
            g = rec.tile([P, 2, 4, BL], dt.float32, tag="gp")
            nc.vector.tensor_tensor(g[:, :, :, :], psg[:, :, :, :],
                                    a_skew(t0), op=OP.mult)
            nc.vector.tensor_tensor(g[:, :, :, :], g[:, :, :, :],
                                    c_skew(t0), op=OP.add)
            wic = rec.tile([P, 2, 2, BL], dt.float32, tag="wic")
            nc.vector.tensor_tensor(wic[:, :, :, :], c_bcast(cp),
                                    wbif_sb[:, :, :, :], op=OP.mult)
            if2 = rec.tile([P, 2, 2, BL], dt.float32, tag="if2p")
            nc.vector.tensor_tensor(if2[:, :, :, :], wic[:, :, :, :],
                                    g[:, :, 0:2, :], op=OP.add)
            sif = rec.tile([P, 2, 2, BL], dt.float32, tag="sifp")
            nc.scalar.activation(sif[:, :, :, :], if2[:, :, :, :], AF.Sigmoid)
            tj = rec.tile([P, 2, BL], dt.float32, tag="tjp")
            nc.scalar.activation(tj[:, :, :], g[:, :, 2, :], AF.Tanh)
            u = rec.tile([P, 2, BL], dt.float32, tag="up")
            nc.vector.tensor_tensor(u[:, :, :], sif[:, :, 0, :], tj[:, :, :],
                                    op=OP.mult)
            v = rec.tile([P, 2, BL], dt.float32, tag="vp")
            nc.vector.tensor_tensor(v[:, :, :], sif[:, :, 1, :], cp,
                                    op=OP.mult)
            cn = cpool.tile([P, 2, BL], dt.float32, tag="cn")
            nc.vector.tensor_tensor(cn[:, :, :], u[:, :, :], v[:, :, :],
                                    op=OP.add)
            wo = rec.tile([P, 2, BL], dt.float32, tag="wop")
            nc.vector.tensor_tensor(wo[:, :, :], cn[:, :, :],
                                    wbo_sb[:, :, :], op=OP.mult)
            o2 = rec.tile([P, 2, BL], dt.float32, tag="o2p")
            nc.vector.tensor_tensor(o2[:, :, :], wo[:, :, :], g[:, :, 3, :],
                                    op=OP.add)
            so = rec.tile([P, 2, BL], dt.float32, tag="sop")
            nc.scalar.activation(so[:, :, :], o2[:, :, :], AF.Sigmoid)
            tc_ = rec.tile([P, 2, BL], dt.float32, tag="tcp")
            nc.scalar.activation(tc_[:, :, :], cn[:, :, :], AF.Tanh)
            nc.vector.tensor_tensor(h_skew(t0), so[:, :, :], tc_[:, :, :],
                                    op=OP.mult)
            cpair_prev = cn[:, :, :]
            h_prev[0] = hT[:, 0, t0 * BL:(t0 + 1) * BL]
            h_prev[1] = hT[:, 1, t1 * BL:(t1 + 1) * BL]

        # logits staging: one chunk = 16 n-tiles of 500 -> 8000 cols
        cur_st = [None]

        def emit_logits_ntile(k, n, eng):
            n0 = n * NTW
            tpc = 8 if k == NB - 1 else 16   # n-tiles per out chunk
            chw = tpc * NTW
            q = n // tpc
            if n % tpc == 0:
                cur_st[0] = stage_p.tile([P, CHW], dt.bfloat16, tag="st",
                                         name="st")
            st = cur_st[0]
            c0 = n0 - q * chw
            ps = ps_log.tile([P, NTW], dt.float32)
            nc.tensor.matmul(ps[:, :], hT[:, 1, k * TPB:(k + 1) * TPB],
                             swt_sb[:, n0:n0 + NTW],
                             start=True, stop=not use_smax_bias)
            if use_smax_bias:
                nc.tensor.matmul(ps[:, :], ones1[:, :],
                                 smb_sb[:, n0:n0 + NTW],
                                 start=False, stop=True)
            if eng == 0:
                nc.vector.tensor_copy(st[:, c0:c0 + NTW], ps[:, :])
            else:
                nc.scalar.copy(st[:, c0:c0 + NTW], ps[:, :])
            if n % tpc == tpc - 1:
                nc.sync.dma_start(
                    out=out_d[k * TPB:(k + 1) * TPB, q * chw:(q + 1) * chw],
                    in_=st[:, 0:chw])

        # layer-0 A/C: block 0 upfront, rest deferred into head slots
        emit_ac(0, 0, TPB)

        # ---- pipelined recurrence + logits ----
        NSLOT = T + SKEW
        pending = []
        ne = 0
        for s in range(NSLOT):
            if s < SKEW:
                emit_step_single(0, s, zero_other=(s == SKEW - 1))
            elif s < T:
                emit_pair(s)
            else:
                emit_step_single(1, s - SKEW)
            if s in (1, 4, 7):
                emit_ac(0, (s // 3 + 1) * TPB, TPB)
            # l1 A/C blocklets: blocklet j (tokens 8j..8j+8) after slot 8j+7
            if s >= ABLK - 1 and (s - (ABLK - 1)) % ABLK == 0:
                j = (s - (ABLK - 1)) // ABLK
                if j < T // ABLK:
                    emit_ac(1, j * ABLK * BL, ABLK * BL)
            # logits block k ready after slot 32k+31+SKEW
            if s >= SPB - 1 + SKEW and (s - (SPB - 1) - SKEW) % SPB == 0:
                k = (s - (SPB - 1) - SKEW) // SPB
                if k < NB:
                    pending.extend(((k, n) for n in range(NPT)))
            for _ in range(2 if s % 2 == 0 else 3):
                if ne < len(pending):
                    k, n = pending[ne]
                    emit_logits_ntile(k, n, ne % 2)
                    ne += 1
        while ne < len(pending):
            k, n = pending[ne]
            emit_logits_ntile(k, n, ne % 2)
            ne += 1

    nc.compile()
    return nc


def _prep_inputs(input_data, embedding, Wx, Wh, alpha, beta1, beta2, bias,
                 wi, wf, wo, softmax_w, softmax_b):
    import ml_dtypes
    bf16 = ml_dtypes.bfloat16
    f32 = np.float32
    input_data = np.asarray(input_data, np.int32)
    embedding = np.ascontiguousarray(np.asarray(embedding, f32))
    Wx = np.asarray(Wx, f32)
    Wh = np.asarray(Wh, f32)
    alpha = np.asarray(alpha, f32)
    beta1 = np.asarray(beta1, f32)
    beta2 = np.asarray(beta2, f32)
    bias = np.asarray(bias, f32)
    wi = np.asarray(wi, f32)
    wf = np.asarray(wf, f32)
    wo = np.asarray(wo, f32)
    softmax_w = np.asarray(softmax_w, f32)
    softmax_b = np.asarray(softmax_b, f32)

    gperm = [0, 2, 1, 3]   # reference order i,j,f,o -> device order i,f,j,o

    def permG(a):
        r = a.reshape(*a.shape[:-1], 4, E)
        return np.ascontiguousarray(r[..., gperm, :].reshape(*a.shape))

    Wxp = permG(Wx)
    alp = permG(alpha)
    b1p = permG(beta1)
    Whp = permG(Wh)
    b2p = permG(beta2)
    bp = permG(bias).copy()
    bp[:, E:2 * E] += FORGET_BIAS          # f-chunk in [i|f|j|o] order

    def to_elg(a):
        return np.ascontiguousarray(np.transpose(a, (1, 0, 2)))

    def to_plk(a):
        return np.ascontiguousarray(
            np.transpose(a.reshape(L, 4, E), (2, 0, 1)))

    pep = np.ascontiguousarray(
        np.transpose(np.stack([wi, wf, wo], axis=1), (2, 0, 1)))  # (E, L, 3)
    wbif = np.ascontiguousarray(np.broadcast_to(
        np.transpose(np.stack([wi, wf], axis=1), (2, 0, 1))[:, :, :, None],
        (E, L, 2, BL))).astype(f32)
    wbo = np.ascontiguousarray(np.broadcast_to(
        wo.T[:, :, None], (E, L, BL))).astype(f32)

    swt = np.ascontiguousarray(softmax_w.T)
    use_smax_bias = bool(np.any(softmax_b))

    common = {
        "emb": embedding,
        "wx": to_elg(Wxp).astype(bf16),
        "wh": to_elg(Whp).astype(bf16),
        "alp": to_plk(alp), "b1t": to_plk(b1p),
        "b2t": to_plk(b2p), "bft": to_plk(bp), "pep": pep,
        "wbif": wbif, "wbo": wbo,
        "swt": swt.astype(bf16),
    }
    if use_smax_bias:
        common["smb"] = softmax_b.reshape(1, V)

    tok = np.arange(NTOK)
    tt_, ss_ = tok // BL, tok % BL
    in_maps = []
    for c in range(NCORES):
        flat = input_data[BL * c + ss_, tt_]
        ids_pm = np.ascontiguousarray(flat.reshape(BL, P).T.astype(np.int32))
        in_maps.append({"ids": ids_pm, **common})
    return in_maps, use_smax_bias


def _run(in_maps, use_smax_bias, trace=False, tmpdir=None):
    from concourse.bass_utils import run_bass_kernel_spmd
    key = use_smax_bias
    if key not in _cache:
        _cache[key] = _build(use_smax_bias)
    nc = _cache[key]
    return run_bass_kernel_spmd(nc, in_maps, core_ids=list(range(NCORES)),
                                trace=trace, tmpdir=tmpdir)


def kernel(**inputs):
    in_maps, use_smax_bias = _prep_inputs(**inputs)
    res = _run(in_maps, use_smax_bias, trace=False)
    # device rows are token order (t*BL + s); reference rows are s*T + t
    tok = np.arange(NTOK)
    row = (tok % BL) * T + tok // BL
    out = np.empty((B * T, V), np.float32)
    for c in range(NCORES):
        out[c * NTOK + row] = res.results[c]["out"].astype(np.float32)
    return out


# revision 16
# speedup vs baseline: 1.1094x; 1.0193x over previous
"""CharRNN (2-layer miLSTM + big logits GEMM) Trainium2 kernel.

Sharding: data-parallel over batch across 8 cores (4 sequences each).
Each core runs the full T=128 recurrence for its 4 sequences and then
computes logits for its own 512 tokens over the FULL vocab (no
collectives). Host concatenates + row-permutes the 8 shards.

Layout is "transposed": features on partitions, batch on the free dim.
Layer 1 runs SKEW=12 steps behind layer 0; the two layers' per-step
gate math is fused into single double-width instructions using skewed
access patterns over combined (layer, ...) buffers. Matmul inputs are
bf16 (weights pre-cast on host, h stored bf16); gate math is f32.
Logits are produced in bf16 (PSUM bf16 -> big bf16 staging tiles ->
2 MB DMAs); the host upcasts to f32.
"""

import numpy as np
from contextlib import ExitStack

V, E, L, B, T = 32000, 128, 2, 32, 128
G = 4 * E
P = 128
NCORES = 8
BL = B // NCORES          # 4 sequences per core
NTOK = BL * T             # 512 tokens per core
FORGET_BIAS = 1.0
SKEW = 10                 # layer-1 lag (slots = T + SKEW)
ABLK = 8                  # l1 A/C blocklet size (steps)
NB = 4                    # logits blocks (128 tokens each)
SPB = T // NB             # steps per logits block = 32
TPB = SPB * BL            # tokens per logits block = 128
NTW = 500                 # logits n-tile width
NPT = V // NTW            # 64 n-tiles per block
CHW = 8000                # out DMA chunk width (16 n-tiles)
NCH = V // CHW            # 4 chunks per block

_cache = {}


def _build(use_smax_bias):
    import concourse.bass as bass
    import concourse.tile as tile
    import concourse.mybir as mybir
    from concourse import bacc
    from concourse.bass import IndirectOffsetOnAxis
    from concourse.masks import make_identity

    dt = mybir.dt
    AF = mybir.ActivationFunctionType
    OP = mybir.AluOpType

    nc = bacc.Bacc("TRN2", target_bir_lowering=False, debug=False,
                   num_devices=NCORES)

    ids_d = nc.dram_tensor("ids", (P, BL), dt.int32, kind="ExternalInput")
    emb_d = nc.dram_tensor("emb", (V, E), dt.float32, kind="ExternalInput")
    wx_d = nc.dram_tensor("wx", (P, L, G), dt.bfloat16, kind="ExternalInput")
    alp_d = nc.dram_tensor("alp", (P, L, 4), dt.float32, kind="ExternalInput")
    b1t_d = nc.dram_tensor("b1t", (P, L, 4), dt.float32, kind="ExternalInput")
    wh_d = nc.dram_tensor("wh", (P, L, G), dt.bfloat16, kind="ExternalInput")
    b2t_d = nc.dram_tensor("b2t", (P, L, 4), dt.float32, kind="ExternalInput")
    bft_d = nc.dram_tensor("bft", (P, L, 4), dt.float32, kind="ExternalInput")
    pep_d = nc.dram_tensor("pep", (P, L, 3), dt.float32, kind="ExternalInput")
    wbif_d = nc.dram_tensor("wbif", (P, L, 2, BL), dt.float32,
                            kind="ExternalInput")
    wbo_d = nc.dram_tensor("wbo", (P, L, BL), dt.float32,
                           kind="ExternalInput")
    swt_d = nc.dram_tensor("swt", (P, V), dt.bfloat16, kind="ExternalInput")
    if use_smax_bias:
        smb_d = nc.dram_tensor("smb", (1, V), dt.float32, kind="ExternalInput")
    # rows of out are in device token order (t*BL + s); host un-permutes
    out_d = nc.dram_tensor("out", (NTOK, V), dt.bfloat16,
                           kind="ExternalOutput")

    with tile.TileContext(nc) as tc, ExitStack() as ctx:
        singles = ctx.enter_context(tc.tile_pool(name="singles", bufs=1))
        big = ctx.enter_context(tc.tile_pool(name="big", bufs=1))
        stage_p = ctx.enter_context(tc.tile_pool(name="stage", bufs=3))
        rec = ctx.enter_context(tc.tile_pool(name="rec", bufs=3))
        cpool = ctx.enter_context(tc.tile_pool(name="cpool", bufs=3))
        ps_big = ctx.enter_context(
            tc.tile_pool(name="ps_big", bufs=2, space="PSUM"))
        ps_g = ctx.enter_context(
            tc.tile_pool(name="ps_g", bufs=3, space="PSUM"))
        ps_log = ctx.enter_context(
            tc.tile_pool(name="ps_log", bufs=3, space="PSUM"))

        # ---- static inputs -> SBUF ----
        ids_sb = singles.tile([P, BL], dt.int32)
        nc.sync.dma_start(out=ids_sb[:, :], in_=ids_d[:, :])
        wx_sb = singles.tile([P, L, G], dt.bfloat16)
        nc.sync.dma_start(out=wx_sb[:, :, :], in_=wx_d[:, :, :])
        alp_sb = singles.tile([P, L, 4], dt.float32)
        nc.sync.dma_start(out=alp_sb[:, :, :], in_=alp_d[:, :, :])
        b1t_sb = singles.tile([P, L, 4], dt.float32)
        nc.sync.dma_start(out=b1t_sb[:, :, :], in_=b1t_d[:, :, :])
        wh_sb = singles.tile([P, L, G], dt.bfloat16)
        nc.sync.dma_start(out=wh_sb[:, :, :], in_=wh_d[:, :, :])
        b2t_sb = singles.tile([P, L, 4], dt.float32)
        nc.sync.dma_start(out=b2t_sb[:, :, :], in_=b2t_d[:, :, :])
        bft_sb = singles.tile([P, L, 4], dt.float32)
        nc.sync.dma_start(out=bft_sb[:, :, :], in_=bft_d[:, :, :])
        pep_sb = singles.tile([P, L, 3], dt.float32)
        nc.sync.dma_start(out=pep_sb[:, :, :], in_=pep_d[:, :, :])
        wbif_sb = singles.tile([P, L, 2, BL], dt.float32)
        nc.sync.dma_start(out=wbif_sb[:, :, :, :], in_=wbif_d[:, :, :, :])
        wbo_sb = singles.tile([P, L, BL], dt.float32)
        nc.sync.dma_start(out=wbo_sb[:, :, :], in_=wbo_d[:, :, :])
        swt_sb = singles.tile([P, V], dt.bfloat16)
        for q in range(8):
            nc.sync.dma_start(out=swt_sb[:, q * 4000:(q + 1) * 4000],
                              in_=swt_d[:, q * 4000:(q + 1) * 4000])
        if use_smax_bias:
            smb_sb = singles.tile([1, V], dt.float32)
            nc.sync.dma_start(out=smb_sb[:, :], in_=smb_d[:, :])
            ones1 = singles.tile([1, P], dt.float32)
            nc.vector.memset(ones1[:, :], 1.0)

        ident = singles.tile([P, P], dt.float32)
        make_identity(nc, ident[:, :])

        zeros4 = singles.tile([P, BL], dt.float32)
        nc.vector.memset(zeros4[:, :], 0.0)
        zeros4h = singles.tile([P, BL], dt.bfloat16)
        nc.vector.memset(zeros4h[:, :], 0.0)

        # ---- embedding gather (tokens on partitions) + transpose ----
        x_sb = singles.tile([P, BL, E], dt.float32)
        for m in range(BL):
            nc.gpsimd.indirect_dma_start(
                out=x_sb[:, m, :], out_offset=None,
                in_=emb_d[:, :],
                in_offset=IndirectOffsetOnAxis(ap=ids_sb[:, m:m + 1], axis=0),
            )
        xT = singles.tile([P, NTOK], dt.bfloat16)
        for m in range(BL):
            pst = ps_big.tile([P, P], dt.float32, tag="psac")
            nc.tensor.transpose(pst[:, :], x_sb[:, m, :], ident[:, :])
            nc.scalar.copy(xT[:, m * P:(m + 1) * P], pst[:, :])

        # ---- combined (layer, ...) buffers ----
        a_all = big.tile([P, L, 4, NTOK], dt.float32)
        c_all = big.tile([P, L, 4, NTOK], dt.float32)
        hT = big.tile([P, L, NTOK], dt.bfloat16)

        SKL_A = a_all.ap[1][0] - SKEW * BL    # layer stride minus skew
        SKL_H = hT.ap[1][0] - SKEW * BL

        def a_skew(t):
            return bass.AP(a_all.tensor, a_all.offset + t * BL,
                           [a_all.ap[0], [SKL_A, 2], a_all.ap[2], [1, BL]])

        def c_skew(t):
            return bass.AP(c_all.tensor, c_all.offset + t * BL,
                           [c_all.ap[0], [SKL_A, 2], c_all.ap[2], [1, BL]])

        def h_skew(t):
            return bass.AP(hT.tensor, hT.offset + t * BL,
                           [hT.ap[0], [SKL_H, 2], [1, BL]])

        def c_bcast(cp):  # (P, 2, BL) pair-c -> (P, 2, 2, BL), dup gate dim
            return bass.AP(cp.tensor, cp.offset,
                           [cp.ap[0], cp.ap[1], [0, 2], cp.ap[2]])

        def emit_ac(l, tok0, ntok):
            src = xT if l == 0 else hT[:, 0, :]
            blk = slice(tok0, tok0 + ntok)
            for k in range(4):
                psx = ps_big.tile([P, TPB], dt.float32, tag="psac")
                nc.tensor.matmul(psx[:, 0:ntok],
                                 wx_sb[:, l, k * P:(k + 1) * P],
                                 src[:, blk])
                nc.vector.tensor_scalar(
                    out=a_all[:, l, k, blk], in0=psx[:, 0:ntok],
                    scalar1=alp_sb[:, l, k:k + 1],
                    scalar2=b2t_sb[:, l, k:k + 1],
                    op0=OP.mult, op1=OP.add)
                nc.scalar.activation(c_all[:, l, k, blk], psx[:, 0:ntok],
                                     AF.Identity,
                                     bias=bft_sb[:, l, k:k + 1],
                                     scale=b1t_sb[:, l, k:k + 1])

        # recurrence state
        cpair_prev = None          # AP (P, 2, BL): [c0_t, c1_{t-SKEW}]
        h_prev = [zeros4h[:, :], zeros4h[:, :]]

        def emit_step_single(l, t, zero_other=False):
            # one-layer step (pipeline head/tail); state kept in pair tiles
            nonlocal cpair_prev
            tb = slice(t * BL, (t + 1) * BL)
            psg = ps_g.tile([P, 2, 4, BL], dt.float32, tag="psg")
            for k in range(4):
                nc.tensor.matmul(psg[:, l, k, :],
                                 wh_sb[:, l, k * P:(k + 1) * P],
                                 h_prev[l], start=(k == 0), stop=(k == 3),
                                 skip_group_check=True)
            cp = zeros4[:, :] if cpair_prev is None else cpair_prev[:, l, :]
            g = rec.tile([P, 4, BL], dt.float32, tag="g")
            nc.vector.tensor_tensor(g[:, :, :], psg[:, l, :, :],
                                    a_all[:, l, :, tb], op=OP.mult)
            nc.vector.tensor_tensor(g[:, :, :], g[:, :, :],
                                    c_all[:, l, :, tb], op=OP.add)
            if2 = rec.tile([P, 2, BL], dt.float32, tag="if2")
            nc.vector.scalar_tensor_tensor(
                if2[:, 0, :], cp, pep_sb[:, l, 0:1], g[:, 0, :],
                op0=OP.mult, op1=OP.add)
            nc.vector.scalar_tensor_tensor(
                if2[:, 1, :], cp, pep_sb[:, l, 1:2], g[:, 1, :],
                op0=OP.mult, op1=OP.add)
            sif = rec.tile([P, 2, BL], dt.float32, tag="sif")
            nc.scalar.activation(sif[:, :, :], if2[:, :, :], AF.Sigmoid)
            tj = rec.tile([P, BL], dt.float32, tag="tj")
            nc.scalar.activation(tj[:, :], g[:, 2, :], AF.Tanh)
            u = rec.tile([P, BL], dt.float32, tag="u")
            nc.vector.tensor_tensor(u[:, :], sif[:, 0, :], tj[:, :],
                                    op=OP.mult)
            v = rec.tile([P, BL], dt.float32, tag="v")
            nc.vector.tensor_tensor(v[:, :], sif[:, 1, :], cp, op=OP.mult)
            cn = cpool.tile([P, 2, BL], dt.float32, tag="cn")
            nc.vector.tensor_tensor(cn[:, l, :], u[:, :], v[:, :], op=OP.add)
            if zero_other:
                nc.vector.memset(cn[:, 1 - l, :], 0.0)
            o2 = rec.tile([P, BL], dt.float32, tag="o2")
            nc.vector.scalar_tensor_tensor(
                o2[:, :], cn[:, l, :], pep_sb[:, l, 2:3], g[:, 3, :],
                op0=OP.mult, op1=OP.add)
            so = rec.tile([P, BL], dt.float32, tag="so")
            nc.scalar.activation(so[:, :], o2[:, :], AF.Sigmoid)
            tc_ = rec.tile([P, BL], dt.float32, tag="tc")
            nc.scalar.activation(tc_[:, :], cn[:, l, :], AF.Tanh)
            nc.vector.tensor_tensor(hT[:, l, tb], so[:, :], tc_[:, :],
                                    op=OP.mult)
            cpair_prev = cn[:, :, :]
            h_prev[l] = hT[:, l, tb]

        def emit_pair(t0):
            # fused: layer0 step t0 + layer1 step t0-SKEW
            nonlocal cpair_prev
            t1 = t0 - SKEW
            psg = ps_g.tile([P, 2, 4, BL], dt.float32, tag="psg")
            for li, tt_ in ((0, t0), (1, t1)):
                for k in range(4):
                    nc.tensor.matmul(
                        psg[:, li, k, :], wh_sb[:, li, k * P:(k + 1) * P],
                        h_prev[li], start=(li == 0 and k == 0),
                        stop=(li == 1 and k == 3), skip_group_check=True)
            cp = cpair_prev
            g = rec.tile([P, 2, 4, BL], dt.float32, tag="gp")
            nc.vector.tensor_tensor(g[:, :, :, :], psg[:, :, :, :],
                                    a_skew(t0), op=OP.mult)
            nc.vector.tensor_tensor(g[:, :, :, :], g[:, :, :, :],
                                    c_skew(t0), op=OP.add)
            wic = rec.tile([P, 2, 2, BL], dt.float32, tag="wic")
            nc.vector.tensor_tensor(wic[:, :, :, :], c_bcast(cp),
                                    wbif_sb[:, :, :, :], op=OP.mult)
            if2 = rec.tile([P, 2, 2, BL], dt.float32, tag="if2p")
            nc.vector.tensor_tensor(if2[:, :, :, :], wic[:, :, :, :],
                                    g[:, :, 0:2, :], op=OP.add)
            sif = rec.tile([P, 2, 2, BL], dt.float32, tag="sifp")
            nc.scalar.activation(sif[:, :, :, :], if2[:, :, :, :], AF.Sigmoid)
            tj = rec.tile([P, 2, BL], dt.float32, tag="tjp")
            nc.scalar.activation(tj[:, :, :], g[:, :, 2, :], AF.Tanh)
            u = rec.tile([P, 2, BL], dt.float32, tag="up")
            nc.vector.tensor_tensor(u[:, :, :], sif[:, :, 0, :], tj[:, :, :],
                                    op=OP.mult)
            v = rec.tile([P, 2, BL], dt.float32, tag="vp")
            nc.vector.tensor_tensor(v[:, :, :], sif[:, :, 1, :], cp,
                                    op=OP.mult)
            cn = cpool.tile([P, 2, BL], dt.float32, tag="cn")
            nc.vector.tensor_tensor(cn[:, :, :], u[:, :, :], v[:, :, :],
                                    op=OP.add)
            wo = rec.tile([P, 2, BL], dt.float32, tag="wop")
            nc.vector.tensor_tensor(wo[:, :, :], cn[:, :, :],
                                    wbo_sb[:, :, :], op=OP.mult)
            o2 = rec.tile([P, 2, BL], dt.float32, tag="o2p")
            nc.vector.tensor_tensor(o2[:, :, :], wo[:, :, :], g[:, :, 3, :],
                                    op=OP.add)
            so = rec.tile([P, 2, BL], dt.float32, tag="sop")
            nc.scalar.activation(so[:, :, :], o2[:, :, :], AF.Sigmoid)
            tc_ = rec.tile([P, 2, BL], dt.float32, tag="tcp")
            nc.scalar.activation(tc_[:, :, :], cn[:, :, :], AF.Tanh)
            nc.vector.tensor_tensor(h_skew(t0), so[:, :, :], tc_[:, :, :],
                                    op=OP.mult)
            cpair_prev = cn[:, :, :]
            h_prev[0] = hT[:, 0, t0 * BL:(t0 + 1) * BL]
            h_prev[1] = hT[:, 1, t1 * BL:(t1 + 1) * BL]

        # logits staging: one chunk = 16 n-tiles of 500 -> 8000 cols
        cur_st = [None]

        def emit_logits_ntile(k, n, eng):
            n0 = n * NTW
            tpc = 8 if k == NB - 1 else 16   # n-tiles per out chunk
            chw = tpc * NTW
            q = n // tpc
            if n % tpc == 0:
                cur_st[0] = stage_p.tile([P, CHW], dt.bfloat16, tag="st",
                                         name="st")
            st = cur_st[0]
            c0 = n0 - q * chw
            ps = ps_log.tile([P, NTW], dt.float32)
            nc.tensor.matmul(ps[:, :], hT[:, 1, k * TPB:(k + 1) * TPB],
                             swt_sb[:, n0:n0 + NTW],
                             start=True, stop=not use_smax_bias)
            if use_smax_bias:
                nc.tensor.matmul(ps[:, :], ones1[:, :],
                                 smb_sb[:, n0:n0 + NTW],
                                 start=False, stop=True)
            if eng == 0:
                nc.vector.tensor_copy(st[:, c0:c0 + NTW], ps[:, :])
            else:
                nc.scalar.copy(st[:, c0:c0 + NTW], ps[:, :])
            if n % tpc == tpc - 1:
                nc.sync.dma_start(
                    out=out_d[k * TPB:(k + 1) * TPB, q * chw:(q + 1) * chw],
                    in_=st[:, 0:chw])

        # layer-0 A/C: block 0 upfront, rest deferred into head slots
        emit_ac(0, 0, TPB)

        # ---- pipelined recurrence + logits ----
        NSLOT = T + SKEW
        pending = []
        ne = 0
        for s in range(NSLOT):
            if s < SKEW:
                emit_step_single(0, s, zero_other=(s == SKEW - 1))
            elif s < T:
                emit_pair(s)
            else:
                emit_step_single(1, s - SKEW)
            if s in (1, 4, 7):
                emit_ac(0, (s // 3 + 1) * TPB, TPB)
            # l1 A/C blocklets: blocklet j (tokens 8j..8j+8) after slot 8j+7
            if s >= ABLK - 1 and (s - (ABLK - 1)) % ABLK == 0:
                j = (s - (ABLK - 1)) // ABLK
                if j < T // ABLK:
                    emit_ac(1, j * ABLK * BL, ABLK * BL)
            # logits block k ready after slot 32k+31+SKEW
            if s >= SPB - 1 + SKEW and (s - (SPB - 1) - SKEW) % SPB == 0:
                k = (s - (SPB - 1) - SKEW) // SPB
                if k < NB:
                    pending.extend(((k, n) for n in range(NPT)))
            for _ in range(2 if s % 2 == 0 else 3):
                if ne < len(pending):
                    k, n = pending[ne]
                    emit_logits_ntile(k, n, ne % 2)
                    ne += 1
        while ne < len(pending):
            k, n = pending[ne]
            emit_logits_ntile(k, n, ne % 2)
            ne += 1

    nc.compile()
    return nc


def _prep_inputs(input_data, embedding, Wx, Wh, alpha, beta1, beta2, bias,
                 wi, wf, wo, softmax_w, softmax_b):
    import ml_dtypes
    bf16 = ml_dtypes.bfloat16
    f32 = np.float32
    input_data = np.asarray(input_data, np.int32)
    embedding = np.ascontiguousarray(np.asarray(embedding, f32))
    Wx = np.asarray(Wx, f32)
    Wh = np.asarray(Wh, f32)
    alpha = np.asarray(alpha, f32)
    beta1 = np.asarray(beta1, f32)
    beta2 = np.asarray(beta2, f32)
    bias = np.asarray(bias, f32)
    wi = np.asarray(wi, f32)
    wf = np.asarray(wf, f32)
    wo = np.asarray(wo, f32)
    softmax_w = np.asarray(softmax_w, f32)
    softmax_b = np.asarray(softmax_b, f32)

    gperm = [0, 2, 1, 3]   # reference order i,j,f,o -> device order i,f,j,o

    def permG(a):
        r = a.reshape(*a.shape[:-1], 4, E)
        return np.ascontiguousarray(r[..., gperm, :].reshape(*a.shape))

    Wxp = permG(Wx)
    alp = permG(alpha)
    b1p = permG(beta1)
    Whp = permG(Wh)
    b2p = permG(beta2)
    bp = permG(bias).copy()
    bp[:, E:2 * E] += FORGET_BIAS          # f-chunk in [i|f|j|o] order

    def to_elg(a):
        return np.ascontiguousarray(np.transpose(a, (1, 0, 2)))

    def to_plk(a):
        return np.ascontiguousarray(
            np.transpose(a.reshape(L, 4, E), (2, 0, 1)))

    pep = np.ascontiguousarray(
        np.transpose(np.stack([wi, wf, wo], axis=1), (2, 0, 1)))  # (E, L, 3)
    wbif = np.ascontiguousarray(np.broadcast_to(
        np.transpose(np.stack([wi, wf], axis=1), (2, 0, 1))[:, :, :, None],
        (E, L, 2, BL))).astype(f32)
    wbo = np.ascontiguousarray(np.broadcast_to(
        wo.T[:, :, None], (E, L, BL))).astype(f32)

    swt = np.ascontiguousarray(softmax_w.T)
    use_smax_bias = bool(np.any(softmax_b))

    common = {
        "emb": embedding,
        "wx": to_elg(Wxp).astype(bf16),
        "wh": to_elg(Whp).astype(bf16),
        "alp": to_plk(alp), "b1t": to_plk(b1p),
        "b2t": to_plk(b2p), "bft": to_plk(bp), "pep": pep,
        "wbif": wbif, "wbo": wbo,
        "swt": swt.astype(bf16),
    }
    if use_smax_bias:
        common["smb"] = softmax_b.reshape(1, V)

    tok = np.arange(NTOK)
    tt_, ss_ = tok // BL, tok % BL
    in_maps = []
    for c in range(NCORES):
        flat = input_data[BL * c + ss_, tt_]
        ids_pm = np.ascontiguousarray(flat.reshape(BL, P).T.astype(np.int32))
        in_maps.append({"ids": ids_pm, **common})
    return in_maps, use_smax_bias


def _run(in_maps, use_smax_bias, trace=False, tmpdir=None):
    from concourse.bass_utils import run_bass_kernel_spmd
    key = use_smax_bias
    if key not in _cache:
        _cache[key] = _build(use_smax_bias)
    nc = _cache[key]
    return run_bass_kernel_spmd(nc, in_maps, core_ids=list(range(NCORES)),
                                trace=trace, tmpdir=tmpdir)


def kernel(**inputs):
    in_maps, use_smax_bias = _prep_inputs(**inputs)
    res = _run(in_maps, use_smax_bias, trace=False)
    # device rows are token order (t*BL + s); reference rows are s*T + t
    tok = np.arange(NTOK)
    row = (tok % BL) * T + tok // BL
    out = np.empty((B * T, V), np.float32)
    for c in range(NCORES):
        out[c * NTOK + row] = res.results[c]["out"].astype(np.float32)
    return out
